# revision 32
# baseline (speedup 1.0000x reference)
"""Trainium2 Bass kernel for DebiasNtXentLoss (B=4096, D=128, 8 NeuronCores).

v2: trapezoid decomposition.  Core c holds row block c (1024 rows) and a
rotated view of znt covering col blocks c..c+4 (5120 cols).  Families:
  d12 (cols 1024..3072) + d3 (3072..4096): full blocks, computed once for
      the pair; mirror row sums shipped as column sums (ones^T matmuls).
  d4 (4096..5120) and d0 (0..1024): the antipodal / diagonal blocks.  Both
      sides of each pair compute the same matrix (transposed), so each core
      computes only the upper trapezoid at 128-row sub-block granularity
      (row tile a covers cols >= 128a), with column sums over the strict
      upper triangle shipped to credit the mirror rows.  Exact - no halving,
      no double compute: 4.325M exps/core vs 5.243M in v1.
Row sums ride on ACT accum_out for row-uniform units and DVE reduces
elsewhere.  PSUM = 2 hybrid tiles [128,2048] = 1536 slab + 512 cs region,
so cs accumulation never stalls the slab pipeline.  cs chunks are drained
to SBUF by the (otherwise idle) Pool engine.  Host reassembles rowsums and
finishes the O(N*D) tail (pos/self/loss) in f64.
"""

import numpy as np

import concourse.bacc as bacc
import concourse.bass as bass
import concourse.mybir as mybir
import concourse.tile as tile
from concourse.bass_utils import run_bass_kernel_spmd

B = 4096
D = 128
N = 2 * B
NCORES = 8
RPC = N // NCORES      # 1024
NCOL = 5 * RPC         # 5120

TEMPERATURE = 0.5
RHO = 0.1
INV_T = 1.0 / TEMPERATURE
N_NEG = N - 2

F32 = mybir.dt.float32
BF16 = mybir.dt.bfloat16
F8 = mybir.dt.float8e4
AF = mybir.ActivationFunctionType
AX = mybir.AxisListType
DR = mybir.MatmulPerfMode.DoubleRow

_CACHE = {}

SLAB_W = 1536   # max width of a slab PSUM tile (3 banks)


# --------------------------------------------------------------------------
# layout
# --------------------------------------------------------------------------
def make_layout():
    """Units stream in order; each unit is a list of pieces
    (fam, a, col_lo, col_hi) packed contiguously (width <= SLAB_W).
    reduce=accum units are single-row-tile (ACT accum_out); others DVE.
    cs chunks: {L0, mms:[(fam, a, lo, hi)], after: unit index that must
    complete first} - emitted just before the slab mms of unit after+1."""
    units = []
    # d12+d3 merged per-m stream: cols [1024, 4096) = 3072 = 2x1536
    for m in range(8):
        units.append(dict(pieces=[("md", m, 1024, 2560)], accum=(m, 0)))
        units.append(dict(pieces=[("md", m, 2560, 4096)], accum=None))
    # d4 trapezoid (a covers [4096+128a, 5120))
    p4 = lambda a: ("d4", a, 4096 + 128 * a, 5120)
    units.append(dict(pieces=[p4(0), p4(6), p4(7)], accum=None))   # 16: 1408
    units.append(dict(pieces=[p4(1), p4(5)], accum=None))          # 17: 1280
    units.append(dict(pieces=[p4(2), p4(3)], accum=None))          # 18: 1408
    units.append(dict(pieces=[p4(4)], accum=None))                 # 19: 512
    # d0 trapezoid (a covers [128a, 1024)); a0-a6 finish early so the last
    # cs chunks fire before the final mini-units
    p0 = lambda a: ("d0", a, 128 * a, 1024)
    units.append(dict(pieces=[p0(0), p0(5)], accum=None))          # 20: 1408
    units.append(dict(pieces=[p0(1), p0(6)], accum=None))          # 21: 1152
    units.append(dict(pieces=[p0(2), p0(3)], accum=None))          # 22: 1408
    units.append(dict(pieces=[p0(4)], accum=(4, 4)))               # 23: 512
    units.append(dict(pieces=[p0(7)], accum=(7, 4)))               # 24: 128

    for u in units:
        off = 0
        offs = []
        for (_, _, lo, hi) in u["pieces"]:
            offs.append(off)
            off += hi - lo
        u["piece_offs"] = offs
        u["width"] = off
        assert off <= SLAB_W

    # index: when is piece (fam, a) complete?  unit idx
    done = {}
    for i, u in enumerate(units):
        for (fam, a, lo, hi) in u["pieces"]:
            done[(fam, a)] = i

    def ready(mms):
        idxs = []
        for (fam, a, lo, hi) in mms:
            if fam in ("d12", "d3"):
                # md units per m are consecutive (2m: cols<2560, 2m+1: rest)
                idxs.append(done[("md", a)] - (1 if hi <= 2560 else 0))
            else:
                idxs.append(done[(fam, a)])
        return max(idxs)

    cs = []
    for k in range(4):      # d12: L0 = 1024+512k
        L0 = 1024 + 512 * k
        for h in range(2):
            mms = [("d12", m, L0, L0 + 512) for m in range(4 * h, 4 * h + 4)]
            cs.append(dict(L0=L0, mms=mms, after=ready(mms)))
    for k in range(2):      # d3: L0 = 3072+512k
        L0 = 3072 + 512 * k
        for h in range(2):
            mms = [("d3", m, L0, L0 + 512) for m in range(4 * h, 4 * h + 4)]
            cs.append(dict(L0=L0, mms=mms, after=ready(mms)))

    def trap_cs(base, k, a_lo, a_hi, fam):
        L0 = base + 512 * k
        mms = []
        for a in range(a_lo, a_hi):
            lo = max(base + 128 * (a + 1), L0)
            if lo < L0 + 512:
                mms.append((fam, a, lo, L0 + 512))
        return dict(L0=L0, mms=mms, after=ready(mms)) if mms else None

    cs.append(trap_cs(4096, 0, 0, 8, "d4"))
    cs.append(trap_cs(4096, 1, 0, 4, "d4"))
    cs.append(trap_cs(4096, 1, 4, 8, "d4"))
    cs.append(trap_cs(0, 0, 0, 8, "d0"))
    cs.append(trap_cs(0, 1, 0, 4, "d0"))
    cs.append(trap_cs(0, 1, 4, 8, "d0"))
    cs = [c for c in cs if c is not None]
    cs.sort(key=lambda c: c["after"])
    for c in cs:
        # valid range within the 512-wide chunk: PSUM in front of the first
        # mm's range is never written (trapezoid chunks) - host must skip it
        c["v0"] = min(lo for (_, _, lo, hi) in c["mms"]) - c["L0"]
    return units, cs


# DVE reduce slot per (fam-or-md, a): column in rs_slots [128, 8, 5]
def reduce_slot(fam, a, unit_idx):
    return {"md": 1, "d4": 2, "d0": 3}[fam]


# --------------------------------------------------------------------------
# bass program
# --------------------------------------------------------------------------
def _build():
    units, cs_chunks = make_layout()
    n_cs = len(cs_chunks)

    nc = bacc.Bacc("TRN2", target_bir_lowering=False, debug=False)
    znt_dram = nc.dram_tensor("znt", [128, NCOL], BF16, kind="ExternalInput")
    rs_dram = nc.dram_tensor("rs", [128, 8], F32, kind="ExternalOutput")
    cols_dram = nc.dram_tensor("cols", [n_cs, 512], F32, kind="ExternalOutput")

    with tile.TileContext(nc) as tc:
        with (
            tc.tile_pool(name="big", bufs=1) as big,
            tc.tile_pool(name="small", bufs=1) as small,
            tc.tile_pool(name="psum", bufs=2, space=bass.MemorySpace.PSUM) as pp,
            tc.tile_pool(name="cspsum", bufs=2,
                         space=bass.MemorySpace.PSUM) as cp,
        ):
            # znt split into one tile per DMA so matmul deps are exact
            zlh0 = big.tile([128, 128], BF16)    # cols [0,128): m0 lhsT
            zlh1 = big.tile([128, 896], BF16)    # cols [128,1024)
            zm1 = big.tile([128, 1024], BF16)    # cols [1024,2048)
            zm2 = big.tile([128, 1024], BF16)    # cols [2048,3072)
            zd3 = big.tile([128, 1024], BF16)    # cols [3072,4096)
            zd4 = big.tile([128, 1024], BF16)    # cols [4096,5120)
            ZCHUNKS = [(0, zlh0), (128, zlh1), (1024, zm1), (2048, zm2),
                       (3072, zd3), (4096, zd4), (5120, None)]
            et_md = big.tile([128, 8, 3072], F8)      # cols [1024,4096) per m
            et_rag = big.tile([128, 9, 1536], BF16)   # ragged d4/d0 units
            cols_sb = big.tile([128, n_cs, 512], F32)

            def zview(lo, hi):
                """SBUF view of znt cols [lo, hi) - must be within a chunk."""
                for (base, t), (nxt, _) in zip(ZCHUNKS, ZCHUNKS[1:]):
                    if lo >= base and hi <= nxt:
                        return t[:, lo - base:hi - base]
                raise AssertionError(f"range [{lo},{hi}) crosses chunks")

            def zsplit(lo, hi):
                """Split [lo,hi) at chunk boundaries then into <=512 runs."""
                bounds = [b for (b, _) in ZCHUNKS if lo < b < hi]
                out = []
                for s, e in zip([lo] + bounds, bounds + [hi]):
                    for c0 in range(s, e, 512):
                        out.append((c0, min(c0 + 512, e)))
                return out

            # ---- input DMA first, all serialized on the sync HWDGE queue
            # in consumption order: HBM bandwidth is the constraint (8 cores
            # pull concurrently), so parallel queues just make every chunk
            # finish late.  Serial order gets zm1 (first unit) in ~1us.
            nc.sync.dma_start(zlh0[:], znt_dram.ap()[:, 0:128])
            nc.sync.dma_start(zm1[:], znt_dram.ap()[:, 1024:2048])
            nc.sync.dma_start(zm2[:], znt_dram.ap()[:, 2048:3072])
            nc.sync.dma_start(zlh1[:], znt_dram.ap()[:, 128:1024])
            nc.sync.dma_start(zd3[:], znt_dram.ap()[:, 3072:4096])
            nc.sync.dma_start(zd4[:], znt_dram.ap()[:, 4096:5120])

            # exp-table warmup on ACT
            w = small.tile([128, 1], F32)
            nc.vector.memset(w[:], 0.0)
            w2 = small.tile([128, 1], F32)
            nc.scalar.activation(w2[:], w[:], AF.Exp)

            ones = small.tile([128, 128], BF16)
            nc.vector.memset(ones[:], 1.0)
            ones8 = small.tile([128, 2, 128], F8)
            nc.vector.memset(ones8[:], 1.0)
            rs_slots = small.tile([128, 8, 5], F32)
            nc.vector.memset(rs_slots[:], 0.0)

            # map (fam, a) -> (et tensor, slot, slot_off, col_lo)
            et_where = {}
            rag_slot = 0

            # warmup matmuls: ramp PE pstate during the DMA wait.  They
            # write garbage into a cs-pool tile; the real cs chunk that
            # later takes this rotation slot resets it with start=True.
            wt = cp.tile([128, 512], F32, tag="cs")
            for _ in range(20):
                nc.tensor.matmul(wt[:, 0:128], ones[:], ones[:],
                                 start=True, stop=True,
                                 skip_group_check=True)

            # cs chunks grouped by the unit they fire after
            cs_by_after = {}
            for j, ch in enumerate(cs_chunks):
                cs_by_after.setdefault(ch["after"], []).append((j, ch))

            def emit_cs(j, ch, copy_eng="vector", pool="cs"):
                """cs matmuls into a cs-pool PSUM tile + drain to SBUF.
                d12/d3 chunks: 4 row tiles as 2 fp8 DoubleRow pair-matmuls
                (contraction over 256 = 2x128 rows).  Trapezoid chunks:
                plain bf16 partial matmuls."""
                ct = (pp if pool == "slab" else cp).tile(
                    [128, 512], F32, tag=("slab" if pool == "slab" else "cs"))
                mms = ch["mms"]
                if mms[0][0] in ("d12", "d3"):
                    m0 = mms[0][1]
                    _, _, lo, hi = mms[0]
                    for i in range(2):
                        m = m0 + 2 * i
                        nc.tensor.matmul(
                            ct[:, 0:512],
                            ones8[:],
                            et_md[:, m:m + 2, lo - 1024:hi - 1024],
                            start=(i == 0), stop=(i == 1),
                            perf_mode=DR, skip_group_check=True,
                        )
                else:
                    for i, (fam, a, lo, hi) in enumerate(mms):
                        tens, slot, soff, base = et_where[(fam, a)]
                        rhs = tens[:, slot, soff + (lo - base):soff + (hi - base)]
                        o0 = lo - ch["L0"]
                        nc.tensor.matmul(
                            ct[:, o0:o0 + (hi - lo)], ones[:], rhs,
                            start=(i == 0), stop=(i == len(mms) - 1),
                            skip_group_check=True,
                        )
                # late chunks drain on ACT (its exp stream is over by then);
                # mid-stream chunks drain on DVE to keep ACT exp-only
                if copy_eng == "scalar" or ch["after"] >= 19:
                    nc.scalar.copy(cols_sb[:, j, :], ct[:, 0:512])
                else:
                    nc.vector.tensor_copy(cols_sb[:, j, :], ct[:, 0:512])

            pending_cs = []
            deferred_reduces = []

            for ui, u in enumerate(units):
                pt = pp.tile([128, SLAB_W], F32, tag="slab")
                # slab matmuls for this unit's pieces
                for (fam, a, lo, hi), poff in zip(u["pieces"], u["piece_offs"]):
                    lhsT = zlh0[:] if a == 0 else zlh1[:, (a - 1) * 128:a * 128]
                    for (c0, c1) in zsplit(lo, hi):
                        nc.tensor.matmul(
                            pt[:, poff + c0 - lo:poff + c1 - lo],
                            lhsT, zview(c0, c1),
                            start=True, stop=True)
                # a cs chunk fires once its source ets are done (ACT runs a
                # unit or two behind PE)
                if pending_cs and ui >= pending_cs[0][1]["after"] + 1:
                    j, ch = pending_cs.pop(0)
                    emit_cs(j, ch)
                # destination in SBUF for the exp tile
                fam0 = u["pieces"][0][0]
                if fam0 == "md":
                    m = u["pieces"][0][1]
                    half = 0 if u["pieces"][0][2] == 1024 else 1536
                    out = et_md[:, m, half:half + u["width"]]
                else:
                    out = et_rag[:, rag_slot, 0:u["width"]]
                    for (fam, a, lo, hi), poff in zip(u["pieces"],
                                                     u["piece_offs"]):
                        et_where[(fam, a)] = (et_rag, rag_slot, poff, lo)
                    rag_slot += 1
                # exp
                acc = u["accum"]
                if acc is not None:
                    m_, k_ = acc
                    nc.scalar.activation(
                        out, pt[:, 0:u["width"]], AF.Exp, scale=INV_T,
                        accum_out=rs_slots[:, m_, k_:k_ + 1])
                else:
                    nc.scalar.activation(
                        out, pt[:, 0:u["width"]], AF.Exp, scale=INV_T)
                    # DVE reduce per piece.  d0-phase reduces are deferred to
                    # the end so the tail cs chunks' PSUM drains aren't stuck
                    # behind them in the DVE queue.
                    for (fam, a, lo, hi), poff in zip(u["pieces"],
                                                     u["piece_offs"]):
                        k_ = reduce_slot(fam0 if fam0 == "md" else fam, a, ui)
                        if fam0 == "md":
                            src = et_md[:, a:a + 1, half:half + u["width"]]
                        else:
                            src = et_rag[:, rag_slot - 1:rag_slot,
                                         poff:poff + (hi - lo)]
                        if ui >= 20:
                            deferred_reduces.append((a, k_, src))
                        else:
                            nc.vector.reduce_sum(
                                rs_slots[:, a, k_:k_ + 1], src, axis=AX.X)
                        if fam0 == "md":
                            break  # single reduce covers the whole md unit
                # queue up cs chunks unlocked by this unit
                for item in cs_by_after.get(ui, []):
                    pending_cs.append(item)

            # leftover cs chunks (fire after the final units); ACT is idle
            # by now so it does the PSUM->SBUF drains, and the freed slab
            # pool slots give the tail chunks a 4-deep rotation
            lpool = ["slab", "cs"]
            for i in range(len(pending_cs)):
                j, ch = pending_cs.pop(0)
                emit_cs(j, ch, copy_eng="scalar", pool=lpool[i % 2])

            # deferred d0-phase reduces run on DVE in parallel with the
            # tail cs drains
            for (a, k_, src) in deferred_reduces:
                nc.vector.reduce_sum(rs_slots[:, a, k_:k_ + 1], src,
                                     axis=AX.X)

            # ---- finalize row sums + outputs ----
            rs = small.tile([128, 8], F32)
            nc.vector.reduce_sum(rs[:], rs_slots[:], axis=AX.X)
            nc.sync.dma_start(rs_dram.ap(), rs[:])
            nc.sync.dma_start(cols_dram.ap(), cols_sb[0:1, :, :])

    nc.compile()
    return nc, cs_chunks


def _get_nc():
    if "nc" not in _CACHE:
        _CACHE["nc"] = _build()
    return _CACHE["nc"]


def _prep_inputs(z_i, z_j):
    import ml_dtypes

    z = np.concatenate(
        [np.asarray(z_i, np.float32), np.asarray(z_j, np.float32)], axis=0
    )
    zn = z / np.maximum(
        np.sqrt((z * z).sum(axis=1, keepdims=True, dtype=np.float32)), 1e-8
    ).astype(np.float32)
    znt = np.ascontiguousarray(zn.T).astype(ml_dtypes.bfloat16)  # [128, 8192]
    in_maps = []
    for c in range(NCORES):
        znt_c = np.roll(znt, -c * RPC, axis=1)[:, :NCOL]
        in_maps.append({"znt": np.ascontiguousarray(znt_c)})
    return in_maps, zn


def kernel(z_i, z_j, _want_results=False, **run_kwargs):
    nc, cs_chunks = _get_nc()
    in_maps, zn = _prep_inputs(z_i, z_j)
    out = run_bass_kernel_spmd(
        nc, in_maps, core_ids=list(range(NCORES)), **run_kwargs
    )
    rowsum = np.zeros(N, dtype=np.float64)
    for c in range(NCORES):
        r = out.results[c]
        rowsum[c * RPC:(c + 1) * RPC] += r["rs"].T.reshape(-1).astype(np.float64)
        cols = r["cols"].astype(np.float64)
        for j, ch in enumerate(cs_chunks):
            g0 = (c * RPC + ch["L0"]) % N
            v0 = ch["v0"]
            rowsum[g0 + v0:g0 + 512] += cols[j][v0:]

    zn64 = zn.astype(np.float64)
    pos = np.exp(INV_T * np.sum(zn64 * np.roll(zn64, -B, axis=0), axis=1))
    slf = np.exp(INV_T * np.sum(zn64 * zn64, axis=1))
    neg = rowsum - slf - pos
    ng = (-RHO * N_NEG * pos + neg) / (1.0 - RHO)
    ng = np.maximum(ng, N_NEG * np.exp(-1.0 / TEMPERATURE))
    losses = np.log(pos + ng) - np.log(pos)
    loss = np.float32(losses.mean())
    if _want_results:
        return loss, out
    return loss


# revision 57
# speedup vs baseline: 1.1122x; 1.1122x over previous
"""Trainium2 Bass kernel for DebiasNtXentLoss (B=4096, D=128, 8 NeuronCores).

v2: trapezoid decomposition.  Core c holds row block c (1024 rows) and a
rotated view of znt covering col blocks c..c+4 (5120 cols).  Families:
  d12 (cols 1024..3072) + d3 (3072..4096): full blocks, computed once for
      the pair; mirror row sums shipped as column sums (ones^T matmuls).
  d4 (4096..5120) and d0 (0..1024): the antipodal / diagonal blocks.  Both
      sides of each pair compute the same matrix (transposed), so each core
      computes only the upper trapezoid at 128-row sub-block granularity
      (row tile a covers cols >= 128a), with column sums over the strict
      upper triangle shipped to credit the mirror rows.  Exact - no halving,
      no double compute: 4.325M exps/core vs 5.243M in v1.
Row sums ride on ACT accum_out for row-uniform units and DVE reduces
elsewhere.  PSUM = 2 hybrid tiles [128,2048] = 1536 slab + 512 cs region,
so cs accumulation never stalls the slab pipeline.  cs chunks are drained
to SBUF by the (otherwise idle) Pool engine.  Host reassembles rowsums and
finishes the O(N*D) tail (pos/self/loss) in f64.
"""

import numpy as np

import concourse.bacc as bacc
import concourse.bass as bass
import concourse.mybir as mybir
import concourse.tile as tile
from concourse.bass_utils import run_bass_kernel_spmd

B = 4096
D = 128
N = 2 * B
NCORES = 8
RPC = N // NCORES      # 1024
NCOL = 5 * RPC         # 5120

TEMPERATURE = 0.5
RHO = 0.1
INV_T = 1.0 / TEMPERATURE
N_NEG = N - 2

F32 = mybir.dt.float32
BF16 = mybir.dt.bfloat16
F8 = mybir.dt.float8e4
AF = mybir.ActivationFunctionType
AX = mybir.AxisListType
DR = mybir.MatmulPerfMode.DoubleRow

_CACHE = {}

SLAB_W = 1536   # max width of a slab PSUM tile (3 banks)


# --------------------------------------------------------------------------
# layout
# --------------------------------------------------------------------------
def make_layout():
    """Units stream in order; each unit is a list of pieces
    (fam, a, col_lo, col_hi) packed contiguously (width <= SLAB_W).
    reduce=accum units are single-row-tile (ACT accum_out); others DVE.
    cs chunks: {L0, mms:[(fam, a, lo, hi)], after: unit index that must
    complete first} - emitted just before the slab mms of unit after+1."""
    units = []
    # d0-a0/a1 first: they only need the first DMA chunk, so the exp
    # stream starts ~2us before the d12 data lands
    p0 = lambda a: ("d0", a, 128 * a, 1024)
    units.append(dict(pieces=[p0(0)], accum=(0, 3)))               # 0
    units.append(dict(pieces=[p0(1)], accum=(1, 3)))               # 1
    # d12+d3 merged per-m stream: cols [1024, 4096) = 3072 = 2x1536.
    # Row sums via ACT accum_out: a DVE reduce of the f8 et here would race
    # ACT's SBUF write (sem fires before the write acks).
    for m in range(8):
        units.append(dict(pieces=[("md", m, 1024, 2560)], accum=(m, 0)))
        units.append(dict(pieces=[("md", m, 2560, 4096)], accum=(m, 1)))
    # d4 trapezoid (a covers [4096+128a, 5120)); row sums by DVE reduce of
    # the PE-written PSUM slab (safe), not of the ACT-written et
    p4 = lambda a: ("d4", a, 4096 + 128 * a, 5120)
    units.append(dict(pieces=[p4(0), p4(6), p4(7)], accum=None))   # 18: 1408
    units.append(dict(pieces=[p4(1), p4(5)], accum=None))          # 19: 1280
    units.append(dict(pieces=[p4(2), p4(3)], accum=None))          # 20: 1408
    units.append(dict(pieces=[p4(4)], accum=None))                 # 21: 512
    # d0 trapezoid tail, PSUM-reduced; a7 last and tiny
    units.append(dict(pieces=[p0(2), p0(3)], accum=None))          # 22: 1408
    units.append(dict(pieces=[p0(4), p0(5), p0(6)], accum=None))   # 23: 1152
    units.append(dict(pieces=[p0(7)], accum=(7, 3)))               # 24: 128

    for u in units:
        off = 0
        offs = []
        for (_, _, lo, hi) in u["pieces"]:
            offs.append(off)
            off += hi - lo
        u["piece_offs"] = offs
        u["width"] = off
        assert off <= SLAB_W

    # index: when is piece (fam, a) complete?  unit idx
    done = {}
    for i, u in enumerate(units):
        for (fam, a, lo, hi) in u["pieces"]:
            done[(fam, a)] = i

    def ready(mms):
        idxs = []
        for (fam, a, lo, hi) in mms:
            if fam in ("d12", "d3"):
                # md units per m are consecutive (2m: cols<2560, 2m+1: rest)
                idxs.append(done[("md", a)] - (1 if hi <= 2560 else 0))
            else:
                idxs.append(done[(fam, a)])
        return max(idxs)

    cs = []
    for k in range(4):      # d12: L0 = 1024+512k
        L0 = 1024 + 512 * k
        for h in range(2):
            mms = [("d12", m, L0, L0 + 512) for m in range(4 * h, 4 * h + 4)]
            cs.append(dict(L0=L0, mms=mms, after=ready(mms)))
    for k in range(2):      # d3: L0 = 3072+512k
        L0 = 3072 + 512 * k
        for h in range(2):
            mms = [("d3", m, L0, L0 + 512) for m in range(4 * h, 4 * h + 4)]
            cs.append(dict(L0=L0, mms=mms, after=ready(mms)))

    def trap_cs(base, k, a_lo, a_hi, fam):
        L0 = base + 512 * k
        mms = []
        for a in range(a_lo, a_hi):
            lo = max(base + 128 * (a + 1), L0)
            if lo < L0 + 512:
                mms.append((fam, a, lo, L0 + 512))
        return dict(L0=L0, mms=mms, after=ready(mms)) if mms else None

    cs.append(trap_cs(4096, 0, 0, 8, "d4"))
    cs.append(trap_cs(4096, 1, 0, 4, "d4"))
    cs.append(trap_cs(4096, 1, 4, 8, "d4"))
    cs.append(trap_cs(0, 0, 0, 8, "d0"))
    cs.append(trap_cs(0, 1, 0, 4, "d0"))
    cs.append(trap_cs(0, 1, 4, 8, "d0"))
    cs = [c for c in cs if c is not None]
    cs.sort(key=lambda c: c["after"])
    for c in cs:
        # valid range within the 512-wide chunk: PSUM in front of the first
        # mm's range is never written (trapezoid chunks) - host must skip it
        c["v0"] = min(lo for (_, _, lo, hi) in c["mms"]) - c["L0"]
    return units, cs


# DVE reduce slot per family: row in rs_slots [128, 4, 8]
def reduce_slot(fam, a, unit_idx):
    return {"md": 1, "d4": 2, "d0": 3}[fam]


# --------------------------------------------------------------------------
# bass program
# --------------------------------------------------------------------------
def _build():
    units, cs_chunks = make_layout()
    n_cs = len(cs_chunks)

    nc = bacc.Bacc("TRN2", target_bir_lowering=False, debug=False)
    znt_dram = nc.dram_tensor("znt", [128, NCOL], BF16, kind="ExternalInput")
    rs_dram = nc.dram_tensor("rs", [128, 8], F32, kind="ExternalOutput")
    rs3_dram = nc.dram_tensor("rs3", [128, 8], F32, kind="ExternalOutput")
    cols_dram = nc.dram_tensor("cols", [n_cs, 512], F32, kind="ExternalOutput")

    with tile.TileContext(nc) as tc:
        with (
            tc.tile_pool(name="big", bufs=1) as big,
            tc.tile_pool(name="small", bufs=1) as small,
            tc.tile_pool(name="psum", bufs=2, space=bass.MemorySpace.PSUM) as pp,
            tc.tile_pool(name="cspsum", bufs=2,
                         space=bass.MemorySpace.PSUM) as cp,
        ):
            # znt split into one tile per DMA so matmul deps are exact
            zlh = big.tile([128, 1024], BF16)    # cols [0,1024): lhsT + d0
            zm1 = big.tile([128, 1024], BF16)    # cols [1024,2048)
            zm2 = big.tile([128, 1024], BF16)    # cols [2048,3072)
            zd3 = big.tile([128, 1024], BF16)    # cols [3072,4096)
            zd4 = big.tile([128, 1024], BF16)    # cols [4096,5120)
            ZCHUNKS = [(0, zlh), (1024, zm1), (2048, zm2),
                       (3072, zd3), (4096, zd4), (5120, None)]
            et_md = big.tile([128, 8, 3072], F8)      # cols [1024,4096) per m
            et_rag = big.tile([128, 12, 1536], BF16)  # ragged d4/d0 units
            cols_sb = big.tile([128, n_cs, 512], F32)

            def zview(lo, hi):
                """SBUF view of znt cols [lo, hi) - must be within a chunk."""
                for (base, t), (nxt, _) in zip(ZCHUNKS, ZCHUNKS[1:]):
                    if lo >= base and hi <= nxt:
                        return t[:, lo - base:hi - base]
                raise AssertionError(f"range [{lo},{hi}) crosses chunks")

            def zsplit(lo, hi):
                """Split [lo,hi) at chunk boundaries then into <=512 runs."""
                bounds = [b for (b, _) in ZCHUNKS if lo < b < hi]
                out = []
                for s, e in zip([lo] + bounds, bounds + [hi]):
                    for c0 in range(s, e, 512):
                        out.append((c0, min(c0 + 512, e)))
                return out

            # ---- input DMA first, all serialized on the sync HWDGE queue
            # in consumption order: HBM bandwidth is the constraint (8 cores
            # pull concurrently), so parallel queues just make every chunk
            # finish late.  Serial order gets zm1 (first unit) in ~1us.
            nc.sync.dma_start(zlh[:], znt_dram.ap()[:, 0:1024])
            nc.sync.dma_start(zm1[:], znt_dram.ap()[:, 1024:2048])
            nc.sync.dma_start(zm2[:], znt_dram.ap()[:, 2048:3072])
            nc.sync.dma_start(zd3[:], znt_dram.ap()[:, 3072:4096])
            nc.sync.dma_start(zd4[:], znt_dram.ap()[:, 4096:5120])

            # exp-table warmup on ACT; the throwaway accum_out also flushes
            # whatever junk the activation accumulator holds at powerup so
            # the first real accum_out unit reads a clean accumulator
            w = small.tile([128, 1], F32)
            nc.vector.memset(w[:], 0.0)
            w2 = small.tile([128, 1], F32)
            wacc = small.tile([128, 2], F32)
            nc.scalar.activation(w2[:], w[:], AF.Exp, accum_out=wacc[:, 0:1])
            nc.scalar.activation(w2[:], w[:], AF.Exp, accum_out=wacc[:, 1:2])

            ones = small.tile([128, 128], BF16)
            nc.vector.memset(ones[:], 1.0)
            ones8 = small.tile([128, 2, 128], F8)
            nc.vector.memset(ones8[:], 1.0)
            # rs_slots[p, k, m]: k = 0 md-A, 1 md-B, 2 d4, 3 d0.  Slot-major
            # so each k-slice is contiguous for DMA.
            rs_slots = small.tile([128, 4, 8], F32)
            nc.vector.memset(rs_slots[:], 0.0)
            race_pad = small.tile([128, 128], F32)

            # map (fam, a) -> (et tensor, slot, slot_off, col_lo)
            et_where = {}
            rag_slot = 0

            # warmup matmuls: ramp PE pstate during the DMA wait.  They
            # write garbage into a cs-pool tile; the real cs chunk that
            # later takes this rotation slot resets it with start=True.
            wt = cp.tile([128, 512], F32, tag="cs")
            for _ in range(20):
                nc.tensor.matmul(wt[:, 0:128], ones[:], ones[:],
                                 start=True, stop=True,
                                 skip_group_check=True)

            # cs chunks grouped by the unit they fire after
            cs_by_after = {}
            for j, ch in enumerate(cs_chunks):
                cs_by_after.setdefault(ch["after"], []).append((j, ch))

            def emit_cs(j, ch, copy_eng="vector", pool="cs"):
                """cs matmuls into a cs-pool PSUM tile + drain to SBUF.
                d12/d3 chunks: 4 row tiles as 2 fp8 DoubleRow pair-matmuls
                (contraction over 256 = 2x128 rows).  Trapezoid chunks:
                plain bf16 partial matmuls."""
                ct = (pp if pool == "slab" else cp).tile(
                    [128, 512], F32, tag=("slab" if pool == "slab" else "cs"))
                mms = ch["mms"]
                if mms[0][0] in ("d12", "d3"):
                    m0 = mms[0][1]
                    _, _, lo, hi = mms[0]
                    for i in range(2):
                        m = m0 + 2 * i
                        nc.tensor.matmul(
                            ct[:, 0:512],
                            ones8[:],
                            et_md[:, m:m + 2, lo - 1024:hi - 1024],
                            start=(i == 0), stop=(i == 1),
                            perf_mode=DR, skip_group_check=True,
                        )
                else:
                    for i, (fam, a, lo, hi) in enumerate(mms):
                        tens, slot, soff, base = et_where[(fam, a)]
                        rhs = tens[:, slot, soff + (lo - base):soff + (hi - base)]
                        o0 = lo - ch["L0"]
                        nc.tensor.matmul(
                            ct[:, o0:o0 + (hi - lo)], ones[:], rhs,
                            start=(i == 0), stop=(i == len(mms) - 1),
                            skip_group_check=True,
                        )
                if copy_eng == "scalar":
                    nc.scalar.copy(cols_sb[:, j, :], ct[:, 0:512])
                else:
                    nc.vector.tensor_copy(cols_sb[:, j, :], ct[:, 0:512])

            pending_cs = []

            for ui, u in enumerate(units):
                pt = pp.tile([128, SLAB_W], F32, tag="slab")
                # slab matmuls for this unit's pieces
                for (fam, a, lo, hi), poff in zip(u["pieces"], u["piece_offs"]):
                    lhsT = zlh[:, a * 128:(a + 1) * 128]
                    for (c0, c1) in zsplit(lo, hi):
                        nc.tensor.matmul(
                            pt[:, poff + c0 - lo:poff + c1 - lo],
                            lhsT, zview(c0, c1),
                            start=True, stop=True)
                # a cs chunk fires once its source ets are done (ACT runs a
                # unit or two behind PE)
                if pending_cs and ui >= pending_cs[0][1]["after"] + 1:
                    j, ch = pending_cs.pop(0)
                    emit_cs(j, ch)
                # destination in SBUF for the exp tile
                fam0 = u["pieces"][0][0]
                if fam0 == "md":
                    m = u["pieces"][0][1]
                    half = 0 if u["pieces"][0][2] == 1024 else 1536
                    out = et_md[:, m, half:half + u["width"]]
                else:
                    out = et_rag[:, rag_slot, 0:u["width"]]
                    for (fam, a, lo, hi), poff in zip(u["pieces"],
                                                     u["piece_offs"]):
                        et_where[(fam, a)] = (et_rag, rag_slot, poff, lo)
                    rag_slot += 1
                # exp
                acc = u["accum"]
                if acc is not None:
                    m_, k_ = acc
                    nc.scalar.activation(
                        out, pt[:, 0:u["width"]], AF.Exp, scale=INV_T,
                        accum_out=rs_slots[:, k_, m_:m_ + 1])
                else:
                    nc.scalar.activation(
                        out, pt[:, 0:u["width"]], AF.Exp, scale=INV_T)
                    # DVE reduces of the ACT-written et race the write-ack
                    # (sem fires when ACT retires, not when SBUF has the
                    # data).  A throwaway copy with the same dependency
                    # delays the real reduces past the window.
                    nc.vector.tensor_copy(
                        race_pad[:], et_rag[:, rag_slot - 1, 0:128])
                    for (fam, a, lo, hi), poff in zip(u["pieces"],
                                                     u["piece_offs"]):
                        k_ = reduce_slot(fam, a, ui)
                        src = et_rag[:, rag_slot - 1:rag_slot,
                                     poff:poff + (hi - lo)]
                        nc.vector.reduce_sum(
                            rs_slots[:, k_, a:a + 1], src, axis=AX.X)
                # slots 0-2 are complete once the d4 phase ends: reduce and
                # ship them mid-stream so only slot 3 remains for the tail
                if ui == 21:
                    rs012 = small.tile([128, 8], F32)
                    nc.vector.reduce_sum(
                        rs012[:],
                        rs_slots[:, 0:3, :].rearrange("p k m -> p m k"),
                        axis=AX.X)
                    nc.sync.dma_start(rs_dram.ap(), rs012[:])
                # queue up cs chunks unlocked by this unit
                for item in cs_by_after.get(ui, []):
                    pending_cs.append(item)

            # leftover cs chunks, if any (fire after the final units)
            lpool = ["slab", "cs"]
            for i in range(len(pending_cs)):
                j, ch = pending_cs.pop(0)
                emit_cs(j, ch, copy_eng="scalar", pool=lpool[i % 2])

            # ---- remaining outputs: slot 3 raw (host adds) + cols ----
            nc.sync.dma_start(rs3_dram.ap(), rs_slots[:, 3, :])
            nc.sync.dma_start(cols_dram.ap(), cols_sb[0:1, :, :])

    nc.compile()
    return nc, cs_chunks


def _get_nc():
    if "nc" not in _CACHE:
        _CACHE["nc"] = _build()
    return _CACHE["nc"]


def _prep_inputs(z_i, z_j):
    import ml_dtypes

    z = np.concatenate(
        [np.asarray(z_i, np.float32), np.asarray(z_j, np.float32)], axis=0
    )
    zn = z / np.maximum(
        np.sqrt((z * z).sum(axis=1, keepdims=True, dtype=np.float32)), 1e-8
    ).astype(np.float32)
    znt = np.ascontiguousarray(zn.T).astype(ml_dtypes.bfloat16)  # [128, 8192]
    in_maps = []
    for c in range(NCORES):
        znt_c = np.roll(znt, -c * RPC, axis=1)[:, :NCOL]
        in_maps.append({"znt": np.ascontiguousarray(znt_c)})
    return in_maps, zn


def kernel(z_i, z_j, _want_results=False, **run_kwargs):
    nc, cs_chunks = _get_nc()
    in_maps, zn = _prep_inputs(z_i, z_j)
    out = run_bass_kernel_spmd(
        nc, in_maps, core_ids=list(range(NCORES)), **run_kwargs
    )
    rowsum = np.zeros(N, dtype=np.float64)
    for c in range(NCORES):
        r = out.results[c]
        rs_full = r["rs"].astype(np.float64) + r["rs3"].astype(np.float64)
        rowsum[c * RPC:(c + 1) * RPC] += rs_full.T.reshape(-1)
        cols = r["cols"].astype(np.float64)
        for j, ch in enumerate(cs_chunks):
            g0 = (c * RPC + ch["L0"]) % N
            v0 = ch["v0"]
            rowsum[g0 + v0:g0 + 512] += cols[j][v0:]

    zn64 = zn.astype(np.float64)
    pos = np.exp(INV_T * np.sum(zn64 * np.roll(zn64, -B, axis=0), axis=1))
    slf = np.exp(INV_T * np.sum(zn64 * zn64, axis=1))
    neg = rowsum - slf - pos
    ng = (-RHO * N_NEG * pos + neg) / (1.0 - RHO)
    ng = np.maximum(ng, N_NEG * np.exp(-1.0 / TEMPERATURE))
    losses = np.log(pos + ng) - np.log(pos)
    loss = np.float32(losses.mean())
    if _want_results:
        return loss, out
    return loss
